# revision 1
# baseline (speedup 1.0000x reference)
"""Sparse spatio-temporal attention layer, data-parallel over batch across
8 NeuronCores (axon/PJRT). B=16,T=12,N=307,D=256,H=8,HD=32.

Sharding: batch 16 -> 8 cores x 2; masks + projection weights replicated.
Each core computes its full attention shard independently (no collectives);
outputs are concatenated on host.
"""

import numpy as np

B, T, N, D = 16, 12, 307, 256
H = 8
HD = D // H

_PMAP_FN = None
_N_DEV = 0


def _build_pmap():
    global _PMAP_FN, _N_DEV
    if _PMAP_FN is not None:
        return _PMAP_FN
    import jax
    import jax.numpy as jnp

    devs = jax.devices()
    nd = min(8, len(devs))
    # batch must divide evenly across devices
    while B % nd != 0:
        nd -= 1
    _N_DEV = nd

    def shard_fn(q, k, v, fm, Wq, bq, Wk, bk, Wv, bv, Wo, bo):
        # q,k,v: [B/nd, T, N, D] on one core
        qp = q @ Wq + bq
        kp = k @ Wk + bk
        vp = v @ Wv + bv
        b = qp.shape[0]
        qp = qp.reshape(b, T, N, H, HD)
        kp = kp.reshape(b, T, N, H, HD)
        vp = vp.reshape(b, T, N, H, HD)
        scores = jnp.einsum("btnhd,btmhd->bhtnm", qp, kp) / jnp.sqrt(
            jnp.float32(HD)
        )
        scores = jnp.where(fm[None, None, None, :, :], -jnp.inf, scores)
        attn = jax.nn.softmax(scores, axis=-1)
        out = jnp.einsum("bhtnm,btmhd->btnhd", attn, vp).reshape(b, T, N, D)
        return out @ Wo + bo

    _PMAP_FN = jax.pmap(
        shard_fn,
        in_axes=(0, 0, 0, None, None, None, None, None, None, None, None, None),
        devices=devs[:nd],
    )
    return _PMAP_FN


def _kernel_numpy(query, key, value, full_mask, Wq, bq, Wk, bk, Wv, bv, Wo, bo):
    q = query @ Wq + bq
    k = key @ Wk + bk
    v = value @ Wv + bv
    q = q.reshape(B, T, N, H, HD)
    k = k.reshape(B, T, N, H, HD)
    v = v.reshape(B, T, N, H, HD)
    scores = np.einsum("btnhd,btmhd->bhtnm", q, k) / np.sqrt(np.float32(HD))
    scores = np.where(full_mask[None, None, None, :, :], -np.inf, scores)
    scores = scores - scores.max(axis=-1, keepdims=True)
    e = np.exp(scores)
    attn = e / e.sum(axis=-1, keepdims=True)
    out = np.einsum("bhtnm,btmhd->btnhd", attn, v).reshape(B, T, N, D)
    return (out @ Wo + bo).astype(np.float32)


def kernel(query, key, value, geo_mask, sem_mask, Wq, bq, Wk, bk, Wv, bv, Wo, bo):
    query = np.asarray(query, np.float32)
    key = np.asarray(key, np.float32)
    value = np.asarray(value, np.float32)
    full_mask = np.asarray(geo_mask, bool) | np.asarray(sem_mask, bool)
    Wq = np.asarray(Wq, np.float32)
    bq = np.asarray(bq, np.float32)
    Wk = np.asarray(Wk, np.float32)
    bk = np.asarray(bk, np.float32)
    Wv = np.asarray(Wv, np.float32)
    bv = np.asarray(bv, np.float32)
    Wo = np.asarray(Wo, np.float32)
    bo = np.asarray(bo, np.float32)
    try:
        fn = _build_pmap()
        nd = _N_DEV
        bl = B // nd
        qs = query.reshape(nd, bl, T, N, D)
        ks = key.reshape(nd, bl, T, N, D)
        vs = value.reshape(nd, bl, T, N, D)
        out = fn(qs, ks, vs, full_mask, Wq, bq, Wk, bk, Wv, bv, Wo, bo)
        return np.asarray(out).reshape(B, T, N, D).astype(np.float32)
    except Exception:
        return _kernel_numpy(
            query, key, value, full_mask, Wq, bq, Wk, bk, Wv, bv, Wo, bo
        )


if __name__ == "__main__":
    rng = np.random.default_rng(0)
    q = rng.standard_normal((B, T, N, D), np.float32)
    out = kernel(
        q, q, q,
        rng.integers(0, 2, (N, N)).astype(bool),
        rng.integers(0, 2, (N, N)).astype(bool),
        rng.standard_normal((D, D), np.float32) / 16,
        np.zeros(D, np.float32),
        rng.standard_normal((D, D), np.float32) / 16,
        np.zeros(D, np.float32),
        rng.standard_normal((D, D), np.float32) / 16,
        np.zeros(D, np.float32),
        rng.standard_normal((D, D), np.float32) / 16,
        np.zeros(D, np.float32),
    )
    print(out.shape, out.dtype, np.abs(out).mean())

